# revision 50
# baseline (speedup 1.0000x reference)
"""Linear attention (silu+1 feature map) MultiHeadAttention kernel for 8x TRN2.

Sharding: data-parallel over batch (B=8 -> 1 batch element per NeuronCore).

fp8 DoubleRow formulation (all big GEMMs at fp8 2x rate, fp32 PSUM):

  stage 1 (stream 512-token tiles, feature-major x = xT8):
    fT[o,t] = silu(s*(WqT.T @ xT) + s*bq)        f = phi_q - 1, fp8 [P,DC,T]
    g[t,d]  = silu(s*(xT.T @ WkT))               g = phi_k - 1, fp8 [P,32,D]
    csg[d] += ones.T-row reductions of g          (DR matmuls, column form)
  A-GEMM (token-major x = xt8, 2 d-half passes, 8 PSUM banks):
    At[E,d] = sum_t x[t,E]*g[t,d]                 DR fp8
    Asb     = At/32 + csx[E]                      (csx = exact host colsum of x)
  kv assembly (bf16):
    kv_h[e,d] = Wv_h @ Asb[:,d_h] + bv_h (x) csg_h   (+ T*bv_h bias at repack)
    (identity: kv = phi_k^T v = Wv@(colsum_x + g^T x) + bv*(T + colsum_g))
  M stage:
    M_h[d,o] = kv_h.T @ (Wo_h/2)  -> m8 = fp8(2*pm) = fp8(M)
    colsum_M via rowsum(kv) hi/lo bf16 split @ Wo   -> phase-2 bias
  phase 2:
    yT[o,t] = m8.T @ f8 + (colsum_M + bo)         DR fp8; out bf16
    (identity: phi_q @ kv @ Wo = f @ M + colsum(M))

Host: fp8 casts (x*32 both layouts, W.T*1024), Wv.T/Wo.T*0.5 bf16, exact
colsum_x, T*bv, bias prep. Output bf16 -> fp32 on host.
"""

import numpy as np
import ml_dtypes

B, T, D = 8, 4096, 1024
H, DH = 16, 64
SCALE = float(DH ** -0.25)
NCORES = 8
P = 128
DC = D // P          # 8 feature chunks
TT = 512             # token tile (stage 1)
NTT = T // TT        # 8 token tiles
NSUB = TT // P       # 4 sub-tiles of 128 tokens
NG = T // P          # 32 token-major g/x subtiles
XS = 32.0            # x fp8 prescale
WS = 1024.0          # Wq/Wk fp8 prescale
QSCALE = SCALE / (XS * WS)
ASCALE = 1.0 / XS    # Asb descale

_BF16 = ml_dtypes.bfloat16
_F8 = ml_dtypes.float8_e4m3

_CACHE = {}


def _split_multi_waits(nc):
    """walrus in this container only encodes ONE sync-wait command per
    instruction. Hoist extra waits onto injected same-engine NOPs placed
    immediately before the instruction (program order on the engine queue
    makes this semantically identical)."""
    import concourse.mybir as mybir

    n_split = 0
    for fn in nc.m.functions:
        for bb in fn.blocks:
            new = []
            changed = False
            for inst in bb.instructions:
                si = inst.sync_info
                waits = list(si.on_wait) if si is not None else []
                if len(waits) > 1:
                    changed = True
                    for j, w in enumerate(waits[:-1]):
                        nop = mybir.InstNoOp(
                            name=f"{inst.name}-sw{j}", ins=[], outs=[]
                        )
                        nop.engine = inst.engine
                        nop.sync_info = mybir.SyncInfo(
                            on_wait=[w], on_update=[]
                        )
                        new.append(nop)
                        n_split += 1
                    inst.sync_info = mybir.SyncInfo(
                        on_wait=[waits[-1]], on_update=list(si.on_update)
                    )
                new.append(inst)
            if changed:
                bb.instructions = new
    return n_split


def _build_program(debug=False):
    import concourse.bass as bass
    import concourse.mybir as mybir
    from concourse.tile import TileContext, add_dep_helper

    dt = mybir.dt
    AF = mybir.ActivationFunctionType
    DR = mybir.MatmulPerfMode.DoubleRow
    ALU = mybir.AluOpType

    nc = bass.Bass()

    # all inputs host-pre-tiled to SBUF layout: every DMA is 128 descriptors
    # of >=4KB (descriptor generation on the trigger engines is the limiter)
    xT8_d = nc.dram_tensor("xT8", [NTT, P, DC * TT], dt.float8e4, kind="ExternalInput")
    xt8_d = nc.dram_tensor("xt8", [P, NG * D], dt.float8e4, kind="ExternalInput")
    wq8_d = nc.dram_tensor("wq8", [P, DC * D], dt.float8e4, kind="ExternalInput")
    wk8_d = nc.dram_tensor("wk8", [P, DC * D], dt.float8e4, kind="ExternalInput")
    wvb_d = nc.dram_tensor("wvb", [P, DC * D], dt.bfloat16, kind="ExternalInput")
    wob_d = nc.dram_tensor("wob", [P, DC * D], dt.bfloat16, kind="ExternalInput")
    bqs_d = nc.dram_tensor("bqs", [P, DC], dt.float32, kind="ExternalInput")
    bos_d = nc.dram_tensor("bos", [P, DC], dt.float32, kind="ExternalInput")
    csx_d = nc.dram_tensor("csx", [P, DC], dt.float32, kind="ExternalInput")
    tbv_d = nc.dram_tensor("tbv", [P, DC], dt.float32, kind="ExternalInput")
    bvr_d = nc.dram_tensor("bvr", [1, D], dt.bfloat16, kind="ExternalInput")
    yT_d = nc.dram_tensor("yT", [D, T], dt.bfloat16, kind="ExternalOutput")
    if debug:
        f_dump = nc.dram_tensor("f_dump", [P, DC, T], dt.float8e4, kind="ExternalOutput")
        g_dump = nc.dram_tensor("g_dump", [P, NG, D], dt.float8e4, kind="ExternalOutput")
        a_dump = nc.dram_tensor("a_dump", [P, DC, D], dt.bfloat16, kind="ExternalOutput")
        kv_dump = nc.dram_tensor("kv_dump", [P, DC, P], dt.bfloat16, kind="ExternalOutput")
        m_dump = nc.dram_tensor("m_dump", [P, DC, D], dt.float8e4, kind="ExternalOutput")
        csg_dump = nc.dram_tensor("csg_dump", [1, D], dt.bfloat16, kind="ExternalOutput")
        bias_dump = nc.dram_tensor("bias_dump", [P, DC], dt.float32, kind="ExternalOutput")

    with TileContext(nc) as tc:
        with (
            tc.tile_pool(name="weights", bufs=1) as wpool,
            tc.tile_pool(name="fstore", bufs=1) as fpool,
            tc.tile_pool(name="msb", bufs=1) as mpool,
        ):
            # pools that die before phase 2 (g, token-major x, x stream, Asb)
            # are scoped manually so phase 2 can reuse their SBUF for deep
            # y-output buffering
            _g_cm = tc.tile_pool(name="gstore", bufs=1)
            gpool = _g_cm.__enter__()
            _xk_cm = tc.tile_pool(name="xtok", bufs=1)
            xkpool = _xk_cm.__enter__()
            _x_cm = tc.tile_pool(name="xin", bufs=6)
            xpool = _x_cm.__enter__()
            _a_cm = tc.tile_pool(name="asb", bufs=1)
            apool = _a_cm.__enter__()
            # ---- weight / const preload ----
            wq_sb = wpool.tile([P, DC, D], dt.float8e4, tag="wq")
            wk_sb = wpool.tile([P, DC, D], dt.float8e4, tag="wk")
            wv_sb = wpool.tile([P, DC, D], dt.bfloat16, tag="wv")
            wo_sb = wpool.tile([P, DC, D], dt.bfloat16, tag="wo")
            bq_sb = wpool.tile([P, DC], dt.float32, tag="bq")
            bo_sb = wpool.tile([P, DC], dt.float32, tag="bo")
            csx_sb = wpool.tile([P, DC], dt.float32, tag="csx")
            tbv_sb = wpool.tile([P, DC], dt.float32, tag="tbv")
            bvr_sb = wpool.tile([1, D], dt.bfloat16, tag="bvr")

            zz = wpool.tile([1, 640], dt.bfloat16, tag="zz")
            nc.vector.memset(zz[:], 0.0)
            # weights on the sync queue; x tiles on gpsimd; token-major x on
            # the scalar queue — three queues run in parallel at startup.
            nc.sync.dma_start(wq_sb[:], wq8_d[:])
            xt_pre = []
            for half in range(2):
                xt0 = xpool.tile([P, DC, TT], dt.float8e4, tag="xt", name=f"xtpre{half}")
                nc.gpsimd.dma_start(xt0[:], xT8_d[half])
                xt_pre.append(xt0)
            nc.sync.dma_start(bq_sb[:], bqs_d[:])
            nc.sync.dma_start(bo_sb[:], bos_d[:])
            nc.sync.dma_start(wk_sb[:], wk8_d[:])

            # token-major x for the A-GEMM (needed only after stage 1):
            # its DMAs are deferred into the pair loop so they don't hog the
            # (exclusive) DMA engines while stage-1 weights/tiles load
            xtok_sb = xkpool.tile([P, NG, D], dt.float8e4, tag="xtok")
            nc.sync.dma_start(csx_sb[:], csx_d[:])
            nc.sync.dma_start(tbv_sb[:], tbv_d[:])
            nc.sync.dma_start(bvr_sb[:], bvr_d[:])

            f8_sb = fpool.tile([P, DC, T], dt.float8e4, tag="f8")
            g_sb = gpool.tile([P, NG, D], dt.float8e4, tag="g8")
            asb = apool.tile([P, DC, D], dt.bfloat16, tag="asb")
            m8_sb = mpool.tile([P, DC, D], dt.float8e4, tag="m8")
            kv_chunks = []
            for c in range(DC):
                kvc = mpool.tile([P, P], dt.bfloat16, tag=f"kvsb{c}", name=f"kvsb{c}")
                kv_chunks.append(kvc)
            csg_row = mpool.tile([1, D], dt.bfloat16, tag="csgrow")
            rs_f32 = mpool.tile([P, DC], dt.float32, tag="rsf32")
            rs_hi = mpool.tile([P, DC], dt.bfloat16, tag="rshi")
            rs_lo = mpool.tile([P, DC], dt.bfloat16, tag="rslo")
            bias_fin = mpool.tile([P, DC], dt.float32, tag="biasfin")

            for c in range(DC):
                nc.vector.memset(kv_chunks[c][:], 0.0)

            # ================= stage 1: projections =================
            with (
                tc.tile_pool(name="ps_q", bufs=2, space="PSUM") as pq_pool,
                tc.tile_pool(name="ps_k", bufs=2, space="PSUM") as pk_pool,
            ):
                # keep PE p-state warm under the startup DMA shadow
                warm = pk_pool.tile([P, D], dt.float32, tag="pk", name="warm")
                for w in range(36):
                    nc.tensor.matmul(
                        warm[:, 0:128], lhsT=zz[:1, :P], rhs=zz[:1, P : P + 128],
                        start=True, stop=True, skip_group_check=True,
                    )

                first_act = {}

                def _q_section(pair, xts):
                    # f[o-feat, token] = silu(SCALE*q + SCALE*bq), fp8 store
                    for oc in range(DC):
                        ps = pq_pool.tile([P, 2 * TT], dt.float32, tag="psq")
                        for j in range(4):
                            for half in range(2):
                                nc.tensor.matmul(
                                    ps[:, half * TT : (half + 1) * TT],
                                    lhsT=wq_sb[:, 2 * j : 2 * j + 2, oc * P : (oc + 1) * P],
                                    rhs=xts[half][:, 2 * j : 2 * j + 2, :],
                                    start=(j == 0), stop=(j == 3), perf_mode=DR,
                                )
                        a = nc.scalar.activation(
                            f8_sb[:, oc, pair * 1024 : (pair + 1) * 1024], ps[:],
                            AF.Silu, bias=bq_sb[:, oc : oc + 1], scale=QSCALE,
                        )
                        if oc == 0:
                            first_act[pair] = a

                last_mm = [None]

                def _k_section(pair, xts):
                    # g[token, d-feat] = silu(SCALE*k), fp8 store token-major
                    for half in range(2):
                        tt = pair * 2 + half
                        xt = xts[half]
                        for sub in range(NSUB):
                            gs = tt * NSUB + sub     # global 128-token subtile
                            pk = pk_pool.tile([P, D], dt.float32, tag="pk")
                            for j in range(4):
                                for n in range(2):
                                    last_mm[0] = nc.tensor.matmul(
                                        pk[:, n * 512 : (n + 1) * 512],
                                        lhsT=xt[:, 2 * j : 2 * j + 2, sub * P : (sub + 1) * P],
                                        rhs=wk_sb[:, 2 * j : 2 * j + 2, n * 512 : (n + 1) * 512],
                                        start=(j == 0), stop=(j == 3), perf_mode=DR,
                                    )
                            nc.scalar.activation(
                                g_sb[:, gs, :], pk[:], AF.Silu, scale=QSCALE,
                            )

                # dim-1 step must be a multiple of 16B for DoubleRow APs
                ones8 = wpool.tile([P, 2, 16], dt.float8e4, tag="ones8")
                nc.vector.memset(ones8[:], 1.0)

                xt_tiles = {0: xt_pre}

                def _fetch(pairq):
                    xts = []
                    for half in range(2):
                        tt = pairq * 2 + half
                        xt = xpool.tile([P, DC, TT], dt.float8e4, tag="xt")
                        nc.gpsimd.dma_start(xt[:], xT8_d[tt])
                        xts.append(xt)
                    xt_tiles[pairq] = xts

                _fetch(1)
                for pair in range(NTT // 2):
                    if pair + 2 < NTT // 2:
                        _fetch(pair + 2)
                    xts = xt_tiles.pop(pair)
                    _q_section(pair, xts)
                    _k_section(pair, xts)
                    # deferred bulk DMAs, gated on this pair's progress and
                    # sliced <=1MB so x tiles never wait long on the shared
                    # DMA engines
                    anchor = first_act[pair]
                    for ch in (2 * pair, 2 * pair + 1):
                        dma = nc.sync.dma_start(
                            xtok_sb[:, ch * 4 : (ch + 1) * 4, :],
                            xt8_d[:, ch * 4 * D : (ch + 1) * 4 * D],
                        )
                        add_dep_helper(dma.ins, anchor.ins, sync=True,
                                       reason="defer xtok behind stage 1")
                    wsb, wd = (wv_sb, wvb_d) if pair < 2 else (wo_sb, wob_d)
                    hh = pair % 2
                    dma = nc.scalar.dma_start(
                        wsb[:, hh * 4 : (hh + 1) * 4, :],
                        wd[:, hh * 4 * D : (hh + 1) * 4 * D],
                    )
                    add_dep_helper(dma.ins, anchor.ins, sync=True,
                                   reason="defer wv/wo")


            # ================= A-GEMM: At[E,d] = x^T g =================
            with tc.tile_pool(name="ps_a", bufs=1, space="PSUM") as pa_pool:
                pa = [
                    pa_pool.tile([P, 512], dt.float32, tag=f"pa{e}", name=f"pa{e}")
                    for e in range(DC)
                ]
                for dh in range(2):
                    for e in range(DC):
                        for s in range(NG // 2):
                            mm = nc.tensor.matmul(
                                pa[e][:],
                                lhsT=xtok_sb[:, 2 * s : 2 * s + 2, e * P : (e + 1) * P],
                                rhs=g_sb[:, 2 * s : 2 * s + 2, dh * 512 : (dh + 1) * 512],
                                start=(s == 0), stop=(s == NG // 2 - 1),
                                perf_mode=DR,
                            )
                            if dh == 0 and s == 0 and last_mm[0] is not None:
                                add_dep_helper(
                                    mm.ins, last_mm[0].ins, sync=False,
                                    reason="A-GEMM after stage 1",
                                )
                        # Asb = At/32 + csx[E]  (exact colsum_x folded in);
                        # drains alternate ACT/DVE and pipeline behind the
                        # next chunk's matmuls
                        if e % 2 == 0:
                            nc.scalar.activation(
                                asb[:, e, dh * 512 : (dh + 1) * 512], pa[e][:],
                                AF.Identity, bias=csx_sb[:, e : e + 1], scale=ASCALE,
                            )
                        else:
                            nc.vector.tensor_scalar(
                                out=asb[:, e, dh * 512 : (dh + 1) * 512],
                                in0=pa[e][:],
                                scalar1=ASCALE,
                                scalar2=csx_sb[:, e : e + 1],
                                op0=mybir.AluOpType.mult,
                                op1=mybir.AluOpType.add,
                            )

            # ============ kv assembly + M stage + phase-2 bias ============
            with (
                tc.tile_pool(name="ps_kv", bufs=1, space="PSUM") as pkv_pool,
                tc.tile_pool(name="ps_m", bufs=2, space="PSUM") as pm_pool,
                tc.tile_pool(name="ps_bias", bufs=1, space="PSUM") as pb_pool,
                tc.tile_pool(name="ps_csg", bufs=1, space="PSUM") as pcsg_pool,
            ):
                kv_ps = pkv_pool.tile([P, 512], dt.float32, tag="kvacc")
                nc.tensor.matmul(
                    kv_ps[:], lhsT=zz[:1, :P], rhs=zz[:1, P : P + 512],
                    start=True, stop=True, skip_group_check=True,
                )
                # csg[d] = colsum of g, directly in row form (ones-lhsT DR)
                for dh in range(2):
                    csg_ps = pcsg_pool.tile([1, 512], dt.float32, tag=f"csg{dh}",
                                            name=f"csg{dh}")
                    for s in range(NG // 2):
                        nc.tensor.matmul(
                            csg_ps[:],
                            lhsT=ones8[:, 0:2, 0:1],
                            rhs=g_sb[:, 2 * s : 2 * s + 2, dh * 512 : (dh + 1) * 512],
                            start=(s == 0), stop=(s == NG // 2 - 1), perf_mode=DR,
                        )
                    nc.vector.tensor_copy(
                        out=csg_row[0:1, dh * 512 : (dh + 1) * 512], in_=csg_ps[:]
                    )

                # kv_h[e,d] = sum_E Wv[e,E] * Asb[E,d]
                for e in range(DC):
                    for h in range(H):
                        r0 = (h % 2) * 64
                        c0 = (h // 2) * 64
                        nc.tensor.matmul(
                            kv_ps[r0 : r0 + 64, c0 : c0 + 64],
                            lhsT=wv_sb[:, e, h * 64 : (h + 1) * 64],
                            rhs=asb[:, e, h * 64 : (h + 1) * 64],
                            start=False, stop=False, skip_group_check=True,
                        )
                # + bv (x) csg  (rank-1 per head)
                for h in range(H):
                    r0 = (h % 2) * 64
                    c0 = (h // 2) * 64
                    nc.tensor.matmul(
                        kv_ps[r0 : r0 + 64, c0 : c0 + 64],
                        lhsT=bvr_sb[0:1, h * 64 : (h + 1) * 64],
                        rhs=csg_row[0:1, h * 64 : (h + 1) * 64],
                        start=False, stop=(h == H - 1), skip_group_check=True,
                    )
                # repack to block-diag chunks (adding T*bv[e] bias), then
                # immediately M(c) and its m8 drain so PE/ACT stay busy;
                # rowsum/bias matmuls follow
                for c in range(DC):
                    for r0 in (0, 64):
                        nc.scalar.activation(
                            kv_chunks[c][r0 : r0 + 64, r0 : r0 + 64],
                            kv_ps[r0 : r0 + 64, c * 64 : (c + 1) * 64],
                            AF.Identity, bias=tbv_sb[r0 : r0 + 64, c : c + 1],
                            scale=1.0,
                            accum_out=rs_f32[r0 : r0 + 64, c : c + 1],
                        )
                    pm = pm_pool.tile([P, D], dt.float32, tag="pm")
                    for n in range(2):
                        nc.tensor.matmul(
                            pm[:, n * 512 : (n + 1) * 512],
                            lhsT=kv_chunks[c][:],
                            rhs=wo_sb[:, c, n * 512 : (n + 1) * 512],
                            start=True, stop=True,
                        )
                    nc.vector.tensor_scalar_mul(m8_sb[:, c, :], pm[:], 2.0)

                nc.gpsimd.tensor_copy(out=rs_hi[:], in_=rs_f32[:])
                nc.gpsimd.tensor_tensor(
                    rs_lo[:], rs_f32[:], rs_hi[:], mybir.AluOpType.subtract,
                )

                bias_ps = pb_pool.tile([P, DC], dt.float32, tag="biasps")
                for oc in range(DC):
                    for ci in range(2 * DC):
                        c, rs = ci // 2, (rs_hi if ci % 2 == 0 else rs_lo)
                        nc.tensor.matmul(
                            bias_ps[:, oc : oc + 1],
                            lhsT=wo_sb[:, c, oc * P : (oc + 1) * P],
                            rhs=rs[:, c : c + 1],
                            start=(ci == 0), stop=(ci == 2 * DC - 1),
                        )
                # bias_fin = 2*bias_ps + bo   (wo was halved on host)
                nc.vector.tensor_scalar_mul(bias_fin[:], bias_ps[:], 2.0)
                nc.vector.tensor_tensor(
                    bias_fin[:], bias_fin[:], bo_sb[:], mybir.AluOpType.add,
                )


            if debug:
                nc.sync.dma_start(f_dump[:], f8_sb[:])
                nc.sync.dma_start(g_dump[:], g_sb[:])
                nc.sync.dma_start(a_dump[:], asb[:])
                for c in range(DC):
                    nc.sync.dma_start(kv_dump[:, c, :], kv_chunks[c][:])
                nc.sync.dma_start(m_dump[:], m8_sb[:])
                nc.sync.dma_start(csg_dump[:], csg_row[:])
                nc.sync.dma_start(bias_dump[:], bias_fin[:])

            # ================= phase 2: yT = m8.T @ f8 + bias =================
            _a_cm.__exit__(None, None, None)
            _x_cm.__exit__(None, None, None)
            _xk_cm.__exit__(None, None, None)
            _g_cm.__exit__(None, None, None)
            _y_cm = tc.tile_pool(name="yout", bufs=10)
            ypool = _y_cm.__enter__()
            with tc.tile_pool(name="ps_y", bufs=4, space="PSUM") as py_pool:
                for oc in range(DC):
                    for qb in range(4):
                        last = oc == DC - 1 and qb >= 2
                        if last:
                            # final block: two independent [128,512] pieces so
                            # the tail is one small drain chain
                            for i in range(2):
                                pyf = py_pool.tile([P, 512], dt.float32, tag="py")
                                for jj in range(4):
                                    pr = (oc + qb + jj) % 4
                                    nc.tensor.matmul(
                                        pyf[:],
                                        lhsT=m8_sb[:, 2 * pr : 2 * pr + 2, oc * P : (oc + 1) * P],
                                        rhs=f8_sb[:, 2 * pr : 2 * pr + 2,
                                                  qb * 1024 + i * 512 : qb * 1024 + (i + 1) * 512],
                                        start=(jj == 0), stop=(jj == 3), perf_mode=DR,
                                    )
                                ysf = ypool.tile([P, 512], dt.bfloat16, tag="ys")
                                if i == 0:
                                    nc.scalar.activation(
                                        ysf[:], pyf[:],
                                        AF.Identity, bias=bias_fin[:, oc : oc + 1], scale=1.0,
                                    )
                                    nc.sync.dma_start(
                                        yT_d[oc * P : (oc + 1) * P,
                                             qb * 1024 : qb * 1024 + 512],
                                        ysf[:],
                                    )
                                else:
                                    nc.vector.tensor_scalar_add(
                                        ysf[:], pyf[:], bias_fin[:, oc : oc + 1]
                                    )
                                    nc.scalar.dma_start(
                                        yT_d[oc * P : (oc + 1) * P,
                                             qb * 1024 + 512 : (qb + 1) * 1024],
                                        ysf[:],
                                    )
                            continue
                        py = py_pool.tile([P, 1024], dt.float32, tag="py")
                        for jj in range(4):
                            pr = (oc + qb + jj) % 4
                            for i in range(2):
                                nc.tensor.matmul(
                                    py[:, i * 512 : (i + 1) * 512],
                                    lhsT=m8_sb[:, 2 * pr : 2 * pr + 2, oc * P : (oc + 1) * P],
                                    rhs=f8_sb[:, 2 * pr : 2 * pr + 2,
                                              qb * 1024 + i * 512 : qb * 1024 + (i + 1) * 512],
                                    start=(jj == 0), stop=(jj == 3), perf_mode=DR,
                                )
                        ys = ypool.tile([P, 1024], dt.bfloat16, tag="ys")
                        nc.scalar.activation(
                            ys[:, 0:512], py[:, 0:512],
                            AF.Identity, bias=bias_fin[:, oc : oc + 1], scale=1.0,
                        )
                        nc.vector.tensor_scalar_add(
                            ys[:, 512:1024], py[:, 512:1024], bias_fin[:, oc : oc + 1]
                        )
                        q_eng = nc.sync if (oc * 4 + qb) % 2 == 0 else nc.gpsimd
                        q_eng.dma_start(
                            yT_d[oc * P : (oc + 1) * P, qb * 1024 : (qb + 1) * 1024],
                            ys[:],
                        )
            _y_cm.__exit__(None, None, None)
    _split_multi_waits(nc)
    return nc


def _get_program(debug=False):
    key = ("nc", debug)
    if key not in _CACHE:
        _CACHE[key] = _build_program(debug)
    return _CACHE[key]


def _f8(a, prescale):
    return np.clip(a * prescale, -240.0, 240.0).astype(_F8)


def _fm(a):
    """feature-major [P, DC] layout of a [D] vector: out[p, c] = a[c*P + p]"""
    return np.ascontiguousarray(a.astype(np.float32).reshape(DC, P).T)


def _wtile(wt):
    """[D_in, D_out] -> SBUF layout [P, DC*D]: row p holds (chunk, out)."""
    return np.ascontiguousarray(
        wt.reshape(DC, P, D).transpose(1, 0, 2).reshape(P, DC * D)
    )


def _prep_shared(Wq, bq, Wk, Wv, bv, Wo, bo):
    return {
        "wq8": _f8(_wtile(np.ascontiguousarray(Wq.T)), WS),
        "wk8": _f8(_wtile(np.ascontiguousarray(Wk.T)), WS),
        "wvb": _wtile(np.ascontiguousarray(Wv.T)).astype(_BF16),
        "wob": _wtile(np.ascontiguousarray(Wo.T * 0.5)).astype(_BF16),
        "bqs": _fm(SCALE * bq),
        "bos": _fm(bo),
        "tbv": _fm(float(T) * bv),
        "bvr": np.ascontiguousarray(bv.reshape(1, D)).astype(_BF16),
    }


def _run(in_maps, trace=False, debug=False, cores=None, **kw):
    from concourse.bass_utils import run_bass_kernel_spmd

    nc = _get_program(debug)
    if cores is None:
        cores = list(range(NCORES))
    return run_bass_kernel_spmd(nc, in_maps, cores, trace=trace, **kw)


def kernel(x, Wq, bq, Wk, Wv, bv, Wo, bo):
    x = np.asarray(x, dtype=np.float32)
    assert x.shape == (B, T, D), x.shape
    shared = _prep_shared(
        np.asarray(Wq, np.float32), np.asarray(bq, np.float32),
        np.asarray(Wk, np.float32), np.asarray(Wv, np.float32),
        np.asarray(bv, np.float32), np.asarray(Wo, np.float32),
        np.asarray(bo, np.float32),
    )
    in_maps = []
    for b in range(B):
        m = dict(shared)
        xb = x[b]
        xbT = np.ascontiguousarray(xb.T)
        m["xT8"] = _f8(
            xbT.reshape(DC, P, NTT, TT).transpose(2, 1, 0, 3).reshape(NTT, P, DC * TT),
            XS,
        )
        m["xt8"] = _f8(
            xb.reshape(NG, P, D).transpose(1, 0, 2).reshape(P, NG * D), XS
        )
        m["csx"] = _fm(xb.sum(axis=0))
        in_maps.append(m)

    res = _run(in_maps)
    out = np.empty((B, T, D), np.float32)
    for b in range(B):
        out[b] = res.results[b]["yT"].astype(np.float32).T
    return out


# revision 53
# speedup vs baseline: 1.0066x; 1.0066x over previous
"""Linear attention (silu+1 feature map) MultiHeadAttention kernel for 8x TRN2.

Sharding: data-parallel over batch (B=8 -> 1 batch element per NeuronCore).

fp8 DoubleRow formulation (all big GEMMs at fp8 2x rate, fp32 PSUM):

  stage 1 (stream 512-token tiles, feature-major x = xT8):
    fT[o,t] = silu(s*(WqT.T @ xT) + s*bq)        f = phi_q - 1, fp8 [P,DC,T]
    g[t,d]  = silu(s*(xT.T @ WkT))               g = phi_k - 1, fp8 [P,32,D]
    csg[d] += ones.T-row reductions of g          (DR matmuls, column form)
  A-GEMM (token-major x = xt8, 2 d-half passes, 8 PSUM banks):
    At[E,d] = sum_t x[t,E]*g[t,d]                 DR fp8
    Asb     = At/32 + csx[E]                      (csx = exact host colsum of x)
  kv assembly (bf16):
    kv_h[e,d] = Wv_h @ Asb[:,d_h] + bv_h (x) csg_h   (+ T*bv_h bias at repack)
    (identity: kv = phi_k^T v = Wv@(colsum_x + g^T x) + bv*(T + colsum_g))
  M stage:
    M_h[d,o] = kv_h.T @ (Wo_h/2)  -> m8 = fp8(2*pm) = fp8(M)
    colsum_M via rowsum(kv) hi/lo bf16 split @ Wo   -> phase-2 bias
  phase 2:
    yT[o,t] = m8.T @ f8 + (colsum_M + bo)         DR fp8; out bf16
    (identity: phi_q @ kv @ Wo = f @ M + colsum(M))

Host: fp8 casts (x*32 both layouts, W.T*1024), Wv.T/Wo.T*0.5 bf16, exact
colsum_x, T*bv, bias prep. Output bf16 -> fp32 on host.
"""

import numpy as np
import ml_dtypes

B, T, D = 8, 4096, 1024
H, DH = 16, 64
SCALE = float(DH ** -0.25)
NCORES = 8
P = 128
DC = D // P          # 8 feature chunks
TT = 512             # token tile (stage 1)
NTT = T // TT        # 8 token tiles
NSUB = TT // P       # 4 sub-tiles of 128 tokens
NG = T // P          # 32 token-major g/x subtiles
XS = 32.0            # x fp8 prescale
WS = 1024.0          # Wq/Wk fp8 prescale
QSCALE = SCALE / (XS * WS)
ASCALE = 1.0 / XS    # Asb descale

_BF16 = ml_dtypes.bfloat16
_F8 = ml_dtypes.float8_e4m3

_CACHE = {}


def _split_multi_waits(nc):
    """walrus in this container only encodes ONE sync-wait command per
    instruction. Hoist extra waits onto injected same-engine NOPs placed
    immediately before the instruction (program order on the engine queue
    makes this semantically identical)."""
    import concourse.mybir as mybir

    n_split = 0
    for fn in nc.m.functions:
        for bb in fn.blocks:
            new = []
            changed = False
            for inst in bb.instructions:
                si = inst.sync_info
                waits = list(si.on_wait) if si is not None else []
                if len(waits) > 1:
                    changed = True
                    for j, w in enumerate(waits[:-1]):
                        nop = mybir.InstNoOp(
                            name=f"{inst.name}-sw{j}", ins=[], outs=[]
                        )
                        nop.engine = inst.engine
                        nop.sync_info = mybir.SyncInfo(
                            on_wait=[w], on_update=[]
                        )
                        new.append(nop)
                        n_split += 1
                    inst.sync_info = mybir.SyncInfo(
                        on_wait=[waits[-1]], on_update=list(si.on_update)
                    )
                new.append(inst)
            if changed:
                bb.instructions = new
    return n_split


def _build_program(debug=False):
    import concourse.bass as bass
    import concourse.mybir as mybir
    from concourse.tile import TileContext, add_dep_helper

    dt = mybir.dt
    AF = mybir.ActivationFunctionType
    DR = mybir.MatmulPerfMode.DoubleRow
    ALU = mybir.AluOpType

    nc = bass.Bass()

    # all inputs host-pre-tiled to SBUF layout: every DMA is 128 descriptors
    # of >=4KB (descriptor generation on the trigger engines is the limiter)
    xT8_d = nc.dram_tensor("xT8", [NTT, P, DC * TT], dt.float8e4, kind="ExternalInput")
    xt8_d = nc.dram_tensor("xt8", [P, NG * D], dt.float8e4, kind="ExternalInput")
    wq8_d = nc.dram_tensor("wq8", [P, DC * D], dt.float8e4, kind="ExternalInput")
    wk8_d = nc.dram_tensor("wk8", [P, DC * D], dt.float8e4, kind="ExternalInput")
    wvb_d = nc.dram_tensor("wvb", [P, DC * D], dt.bfloat16, kind="ExternalInput")
    wob_d = nc.dram_tensor("wob", [P, DC * D], dt.bfloat16, kind="ExternalInput")
    bqs_d = nc.dram_tensor("bqs", [P, DC], dt.float32, kind="ExternalInput")
    bos_d = nc.dram_tensor("bos", [P, DC], dt.float32, kind="ExternalInput")
    csx_d = nc.dram_tensor("csx", [P, DC], dt.float32, kind="ExternalInput")
    tbv_d = nc.dram_tensor("tbv", [P, DC], dt.float32, kind="ExternalInput")
    bvr_d = nc.dram_tensor("bvr", [1, D], dt.bfloat16, kind="ExternalInput")
    yT_d = nc.dram_tensor("yT", [D, T], dt.bfloat16, kind="ExternalOutput")
    if debug:
        f_dump = nc.dram_tensor("f_dump", [P, DC, T], dt.float8e4, kind="ExternalOutput")
        g_dump = nc.dram_tensor("g_dump", [P, NG, D], dt.float8e4, kind="ExternalOutput")
        a_dump = nc.dram_tensor("a_dump", [P, DC, D], dt.bfloat16, kind="ExternalOutput")
        kv_dump = nc.dram_tensor("kv_dump", [P, DC, P], dt.bfloat16, kind="ExternalOutput")
        m_dump = nc.dram_tensor("m_dump", [P, DC, D], dt.float8e4, kind="ExternalOutput")
        csg_dump = nc.dram_tensor("csg_dump", [1, D], dt.bfloat16, kind="ExternalOutput")
        bias_dump = nc.dram_tensor("bias_dump", [P, DC], dt.float32, kind="ExternalOutput")

    with TileContext(nc) as tc:
        with (
            tc.tile_pool(name="weights", bufs=1) as wpool,
            tc.tile_pool(name="fstore", bufs=1) as fpool,
            tc.tile_pool(name="msb", bufs=1) as mpool,
        ):
            # pools that die before phase 2 (g, token-major x, x stream, Asb)
            # are scoped manually so phase 2 can reuse their SBUF for deep
            # y-output buffering
            _g_cm = tc.tile_pool(name="gstore", bufs=1)
            gpool = _g_cm.__enter__()
            _xk_cm = tc.tile_pool(name="xtok", bufs=1)
            xkpool = _xk_cm.__enter__()
            _x_cm = tc.tile_pool(name="xin", bufs=6)
            xpool = _x_cm.__enter__()
            _a_cm = tc.tile_pool(name="asb", bufs=1)
            apool = _a_cm.__enter__()
            # ---- weight / const preload ----
            wq_sb = wpool.tile([P, DC, D], dt.float8e4, tag="wq")
            wk_sb = wpool.tile([P, DC, D], dt.float8e4, tag="wk")
            wv_sb = wpool.tile([P, DC, D], dt.bfloat16, tag="wv")
            wo_sb = wpool.tile([P, DC, D], dt.bfloat16, tag="wo")
            bq_sb = wpool.tile([P, DC], dt.float32, tag="bq")
            bo_sb = wpool.tile([P, DC], dt.float32, tag="bo")
            csx_sb = wpool.tile([P, DC], dt.float32, tag="csx")
            tbv_sb = wpool.tile([P, DC], dt.float32, tag="tbv")
            bvr_sb = wpool.tile([1, D], dt.bfloat16, tag="bvr")

            zz = wpool.tile([1, 640], dt.bfloat16, tag="zz")
            nc.vector.memset(zz[:], 0.0)
            # weights on the sync queue; x tiles on gpsimd; token-major x on
            # the scalar queue — three queues run in parallel at startup.
            nc.sync.dma_start(wq_sb[:], wq8_d[:])
            xt_pre = []
            for half in range(2):
                xt0 = xpool.tile([P, DC, TT], dt.float8e4, tag="xt", name=f"xtpre{half}")
                nc.gpsimd.dma_start(xt0[:], xT8_d[half])
                xt_pre.append(xt0)
            nc.sync.dma_start(bq_sb[:], bqs_d[:])
            nc.sync.dma_start(bo_sb[:], bos_d[:])
            nc.sync.dma_start(wk_sb[:], wk8_d[:])

            # token-major x for the A-GEMM (needed only after stage 1):
            # its DMAs are deferred into the pair loop so they don't hog the
            # (exclusive) DMA engines while stage-1 weights/tiles load
            xtok_sb = xkpool.tile([P, NG, D], dt.float8e4, tag="xtok")
            nc.sync.dma_start(csx_sb[:], csx_d[:])
            nc.sync.dma_start(tbv_sb[:], tbv_d[:])
            nc.sync.dma_start(bvr_sb[:], bvr_d[:])

            f8_sb = fpool.tile([P, DC, T], dt.float8e4, tag="f8")
            g_sb = gpool.tile([P, NG, D], dt.float8e4, tag="g8")
            asb = apool.tile([P, DC, D], dt.bfloat16, tag="asb")
            m8_sb = mpool.tile([P, DC, D], dt.float8e4, tag="m8")
            kv_chunks = []
            for c in range(DC):
                kvc = mpool.tile([P, P], dt.bfloat16, tag=f"kvsb{c}", name=f"kvsb{c}")
                kv_chunks.append(kvc)
            csg_row = mpool.tile([1, D], dt.bfloat16, tag="csgrow")
            rs_f32 = mpool.tile([P, DC], dt.float32, tag="rsf32")
            rs_hi = mpool.tile([P, DC], dt.bfloat16, tag="rshi")
            rs_lo = mpool.tile([P, DC], dt.bfloat16, tag="rslo")
            bias_fin = mpool.tile([P, DC], dt.float32, tag="biasfin")

            for c in range(DC):
                nc.vector.memset(kv_chunks[c][:], 0.0)

            # ================= stage 1: projections =================
            with (
                tc.tile_pool(name="ps_q", bufs=2, space="PSUM") as pq_pool,
                tc.tile_pool(name="ps_k", bufs=2, space="PSUM") as pk_pool,
            ):
                # keep PE p-state warm under the startup DMA shadow
                warm = pk_pool.tile([P, D], dt.float32, tag="pk", name="warm")
                for w in range(36):
                    nc.tensor.matmul(
                        warm[:, 0:128], lhsT=zz[:1, :P], rhs=zz[:1, P : P + 128],
                        start=True, stop=True, skip_group_check=True,
                    )

                first_act = {}

                def _q_section(pair, xts):
                    # f[o-feat, token] = silu(SCALE*q + SCALE*bq), fp8 store
                    for oc in range(DC):
                        ps = pq_pool.tile([P, 2 * TT], dt.float32, tag="psq")
                        for j in range(4):
                            for half in range(2):
                                nc.tensor.matmul(
                                    ps[:, half * TT : (half + 1) * TT],
                                    lhsT=wq_sb[:, 2 * j : 2 * j + 2, oc * P : (oc + 1) * P],
                                    rhs=xts[half][:, 2 * j : 2 * j + 2, :],
                                    start=(j == 0), stop=(j == 3), perf_mode=DR,
                                )
                        a = nc.scalar.activation(
                            f8_sb[:, oc, pair * 1024 : (pair + 1) * 1024], ps[:],
                            AF.Silu, bias=bq_sb[:, oc : oc + 1], scale=QSCALE,
                        )
                        if oc == 0:
                            first_act[pair] = a

                last_mm = [None]

                def _k_section(pair, xts):
                    # g[token, d-feat] = silu(SCALE*k), fp8 store token-major
                    for half in range(2):
                        tt = pair * 2 + half
                        xt = xts[half]
                        for sub in range(NSUB):
                            gs = tt * NSUB + sub     # global 128-token subtile
                            pk = pk_pool.tile([P, D], dt.float32, tag="pk")
                            for j in range(4):
                                for n in range(2):
                                    last_mm[0] = nc.tensor.matmul(
                                        pk[:, n * 512 : (n + 1) * 512],
                                        lhsT=xt[:, 2 * j : 2 * j + 2, sub * P : (sub + 1) * P],
                                        rhs=wk_sb[:, 2 * j : 2 * j + 2, n * 512 : (n + 1) * 512],
                                        start=(j == 0), stop=(j == 3), perf_mode=DR,
                                    )
                            nc.scalar.activation(
                                g_sb[:, gs, :], pk[:], AF.Silu, scale=QSCALE,
                            )

                # dim-1 step must be a multiple of 16B for DoubleRow APs
                ones8 = wpool.tile([P, 2, 16], dt.float8e4, tag="ones8")
                nc.vector.memset(ones8[:], 1.0)

                xt_tiles = {0: xt_pre}

                def _fetch(pairq):
                    xts = []
                    for half in range(2):
                        tt = pairq * 2 + half
                        xt = xpool.tile([P, DC, TT], dt.float8e4, tag="xt")
                        nc.gpsimd.dma_start(xt[:], xT8_d[tt])
                        xts.append(xt)
                    xt_tiles[pairq] = xts

                _fetch(1)
                for pair in range(NTT // 2):
                    if pair + 2 < NTT // 2:
                        _fetch(pair + 2)
                    xts = xt_tiles.pop(pair)
                    _q_section(pair, xts)
                    _k_section(pair, xts)
                    # deferred bulk DMAs, gated on this pair's progress and
                    # sliced <=1MB so x tiles never wait long on the shared
                    # DMA engines
                    anchor = first_act[pair]
                    for ch in (2 * pair, 2 * pair + 1):
                        dma = nc.sync.dma_start(
                            xtok_sb[:, ch * 4 : (ch + 1) * 4, :],
                            xt8_d[:, ch * 4 * D : (ch + 1) * 4 * D],
                        )
                        add_dep_helper(dma.ins, anchor.ins, sync=True,
                                       reason="defer xtok behind stage 1")
                    wsb, wd = (wv_sb, wvb_d) if pair < 2 else (wo_sb, wob_d)
                    hh = pair % 2
                    dma = nc.scalar.dma_start(
                        wsb[:, hh * 4 : (hh + 1) * 4, :],
                        wd[:, hh * 4 * D : (hh + 1) * 4 * D],
                    )
                    add_dep_helper(dma.ins, anchor.ins, sync=True,
                                   reason="defer wv/wo")


            # ================= A-GEMM: At[E,d] = x^T g =================
            with tc.tile_pool(name="ps_a", bufs=1, space="PSUM") as pa_pool:
                pa = [
                    pa_pool.tile([P, 512], dt.float32, tag=f"pa{e}", name=f"pa{e}")
                    for e in range(DC)
                ]
                for dh in range(2):
                    for e in range(DC):
                        for s in range(NG // 2):
                            mm = nc.tensor.matmul(
                                pa[e][:],
                                lhsT=xtok_sb[:, 2 * s : 2 * s + 2, e * P : (e + 1) * P],
                                rhs=g_sb[:, 2 * s : 2 * s + 2, dh * 512 : (dh + 1) * 512],
                                start=(s == 0), stop=(s == NG // 2 - 1),
                                perf_mode=DR,
                            )
                            if dh == 0 and s == 0 and last_mm[0] is not None:
                                add_dep_helper(
                                    mm.ins, last_mm[0].ins, sync=False,
                                    reason="A-GEMM after stage 1",
                                )
                        # Asb = At/32 + csx[E]  (exact colsum_x folded in);
                        # drains alternate ACT/DVE and pipeline behind the
                        # next chunk's matmuls
                        if e % 2 == 0:
                            nc.scalar.activation(
                                asb[:, e, dh * 512 : (dh + 1) * 512], pa[e][:],
                                AF.Identity, bias=csx_sb[:, e : e + 1], scale=ASCALE,
                            )
                        else:
                            nc.vector.tensor_scalar(
                                out=asb[:, e, dh * 512 : (dh + 1) * 512],
                                in0=pa[e][:],
                                scalar1=ASCALE,
                                scalar2=csx_sb[:, e : e + 1],
                                op0=mybir.AluOpType.mult,
                                op1=mybir.AluOpType.add,
                            )

            # ============ kv assembly + M stage + phase-2 bias ============
            with (
                tc.tile_pool(name="ps_kv", bufs=1, space="PSUM") as pkv_pool,
                tc.tile_pool(name="ps_m", bufs=2, space="PSUM") as pm_pool,
                tc.tile_pool(name="ps_bias", bufs=1, space="PSUM") as pb_pool,
                tc.tile_pool(name="ps_csg", bufs=1, space="PSUM") as pcsg_pool,
            ):
                kv_ps = pkv_pool.tile([P, 512], dt.float32, tag="kvacc")
                nc.tensor.matmul(
                    kv_ps[:], lhsT=zz[:1, :P], rhs=zz[:1, P : P + 512],
                    start=True, stop=True, skip_group_check=True,
                )
                # csg[d] = colsum of g, directly in row form (ones-lhsT DR)
                for dh in range(2):
                    csg_ps = pcsg_pool.tile([1, 512], dt.float32, tag=f"csg{dh}",
                                            name=f"csg{dh}")
                    for s in range(NG // 2):
                        nc.tensor.matmul(
                            csg_ps[:],
                            lhsT=ones8[:, 0:2, 0:1],
                            rhs=g_sb[:, 2 * s : 2 * s + 2, dh * 512 : (dh + 1) * 512],
                            start=(s == 0), stop=(s == NG // 2 - 1), perf_mode=DR,
                        )
                    nc.vector.tensor_copy(
                        out=csg_row[0:1, dh * 512 : (dh + 1) * 512], in_=csg_ps[:]
                    )

                # kv_h[e,d] = sum_E Wv[e,E] * Asb[E,d]
                for e in range(DC):
                    for h in range(H):
                        r0 = (h % 2) * 64
                        c0 = (h // 2) * 64
                        nc.tensor.matmul(
                            kv_ps[r0 : r0 + 64, c0 : c0 + 64],
                            lhsT=wv_sb[:, e, h * 64 : (h + 1) * 64],
                            rhs=asb[:, e, h * 64 : (h + 1) * 64],
                            start=False, stop=False, skip_group_check=True,
                        )
                # + bv (x) csg  (rank-1 per head)
                for h in range(H):
                    r0 = (h % 2) * 64
                    c0 = (h // 2) * 64
                    nc.tensor.matmul(
                        kv_ps[r0 : r0 + 64, c0 : c0 + 64],
                        lhsT=bvr_sb[0:1, h * 64 : (h + 1) * 64],
                        rhs=csg_row[0:1, h * 64 : (h + 1) * 64],
                        start=False, stop=(h == H - 1), skip_group_check=True,
                    )
                # repack to block-diag chunks (adding T*bv[e] bias), then
                # immediately M(c) and its m8 drain so PE/ACT stay busy;
                # rowsum/bias matmuls follow
                for c in range(DC):
                    for r0 in (0, 64):
                        nc.scalar.activation(
                            kv_chunks[c][r0 : r0 + 64, r0 : r0 + 64],
                            kv_ps[r0 : r0 + 64, c * 64 : (c + 1) * 64],
                            AF.Identity, bias=tbv_sb[r0 : r0 + 64, c : c + 1],
                            scale=1.0,
                            accum_out=rs_f32[r0 : r0 + 64, c : c + 1],
                        )
                    pm = pm_pool.tile([P, D], dt.float32, tag="pm")
                    for n in range(2):
                        nc.tensor.matmul(
                            pm[:, n * 512 : (n + 1) * 512],
                            lhsT=kv_chunks[c][:],
                            rhs=wo_sb[:, c, n * 512 : (n + 1) * 512],
                            start=True, stop=True,
                        )
                    if c in (3, 7):
                        nc.scalar.mul(m8_sb[:, c, :], pm[:], 2.0)
                    else:
                        nc.vector.tensor_scalar_mul(m8_sb[:, c, :], pm[:], 2.0)

                nc.gpsimd.tensor_copy(out=rs_hi[:], in_=rs_f32[:])
                nc.gpsimd.tensor_tensor(
                    rs_lo[:], rs_f32[:], rs_hi[:], mybir.AluOpType.subtract,
                )

                bias_ps = pb_pool.tile([P, DC], dt.float32, tag="biasps")
                for oc in range(DC):
                    for ci in range(2 * DC):
                        c, rs = ci // 2, (rs_hi if ci % 2 == 0 else rs_lo)
                        nc.tensor.matmul(
                            bias_ps[:, oc : oc + 1],
                            lhsT=wo_sb[:, c, oc * P : (oc + 1) * P],
                            rhs=rs[:, c : c + 1],
                            start=(ci == 0), stop=(ci == 2 * DC - 1),
                        )
                # bias_fin = 2*bias_ps + bo   (wo was halved on host)
                nc.vector.tensor_scalar_mul(bias_fin[:], bias_ps[:], 2.0)
                nc.vector.tensor_tensor(
                    bias_fin[:], bias_fin[:], bo_sb[:], mybir.AluOpType.add,
                )


            if debug:
                nc.sync.dma_start(f_dump[:], f8_sb[:])
                nc.sync.dma_start(g_dump[:], g_sb[:])
                nc.sync.dma_start(a_dump[:], asb[:])
                for c in range(DC):
                    nc.sync.dma_start(kv_dump[:, c, :], kv_chunks[c][:])
                nc.sync.dma_start(m_dump[:], m8_sb[:])
                nc.sync.dma_start(csg_dump[:], csg_row[:])
                nc.sync.dma_start(bias_dump[:], bias_fin[:])

            # ================= phase 2: yT = m8.T @ f8 + bias =================
            _a_cm.__exit__(None, None, None)
            _x_cm.__exit__(None, None, None)
            _xk_cm.__exit__(None, None, None)
            _g_cm.__exit__(None, None, None)
            _y_cm = tc.tile_pool(name="yout", bufs=10)
            ypool = _y_cm.__enter__()
            with tc.tile_pool(name="ps_y", bufs=4, space="PSUM") as py_pool:
                for oc in range(DC):
                    for qb in range(4):
                        last = oc == DC - 1 and qb >= 2
                        if last:
                            # final block: two independent [128,512] pieces so
                            # the tail is one small drain chain
                            for i in range(2):
                                pyf = py_pool.tile([P, 512], dt.float32, tag="py")
                                for jj in range(4):
                                    pr = (oc + qb + jj) % 4
                                    nc.tensor.matmul(
                                        pyf[:],
                                        lhsT=m8_sb[:, 2 * pr : 2 * pr + 2, oc * P : (oc + 1) * P],
                                        rhs=f8_sb[:, 2 * pr : 2 * pr + 2,
                                                  qb * 1024 + i * 512 : qb * 1024 + (i + 1) * 512],
                                        start=(jj == 0), stop=(jj == 3), perf_mode=DR,
                                    )
                                ysf = ypool.tile([P, 512], dt.bfloat16, tag="ys")
                                if i == 0:
                                    nc.scalar.activation(
                                        ysf[:], pyf[:],
                                        AF.Identity, bias=bias_fin[:, oc : oc + 1], scale=1.0,
                                    )
                                    nc.sync.dma_start(
                                        yT_d[oc * P : (oc + 1) * P,
                                             qb * 1024 : qb * 1024 + 512],
                                        ysf[:],
                                    )
                                else:
                                    nc.vector.tensor_scalar_add(
                                        ysf[:], pyf[:], bias_fin[:, oc : oc + 1]
                                    )
                                    nc.scalar.dma_start(
                                        yT_d[oc * P : (oc + 1) * P,
                                             qb * 1024 + 512 : (qb + 1) * 1024],
                                        ysf[:],
                                    )
                            continue
                        py = py_pool.tile([P, 1024], dt.float32, tag="py")
                        for jj in range(4):
                            pr = (oc + qb + jj) % 4
                            for i in range(2):
                                nc.tensor.matmul(
                                    py[:, i * 512 : (i + 1) * 512],
                                    lhsT=m8_sb[:, 2 * pr : 2 * pr + 2, oc * P : (oc + 1) * P],
                                    rhs=f8_sb[:, 2 * pr : 2 * pr + 2,
                                              qb * 1024 + i * 512 : qb * 1024 + (i + 1) * 512],
                                    start=(jj == 0), stop=(jj == 3), perf_mode=DR,
                                )
                        ys = ypool.tile([P, 1024], dt.bfloat16, tag="ys")
                        nc.scalar.activation(
                            ys[:, 0:512], py[:, 0:512],
                            AF.Identity, bias=bias_fin[:, oc : oc + 1], scale=1.0,
                        )
                        nc.vector.tensor_scalar_add(
                            ys[:, 512:1024], py[:, 512:1024], bias_fin[:, oc : oc + 1]
                        )
                        q_eng = nc.sync if (oc * 4 + qb) % 2 == 0 else nc.gpsimd
                        q_eng.dma_start(
                            yT_d[oc * P : (oc + 1) * P, qb * 1024 : (qb + 1) * 1024],
                            ys[:],
                        )
            _y_cm.__exit__(None, None, None)
    _split_multi_waits(nc)
    return nc


def _get_program(debug=False):
    key = ("nc", debug)
    if key not in _CACHE:
        _CACHE[key] = _build_program(debug)
    return _CACHE[key]


def _f8(a, prescale):
    return np.clip(a * prescale, -240.0, 240.0).astype(_F8)


def _fm(a):
    """feature-major [P, DC] layout of a [D] vector: out[p, c] = a[c*P + p]"""
    return np.ascontiguousarray(a.astype(np.float32).reshape(DC, P).T)


def _wtile(wt):
    """[D_in, D_out] -> SBUF layout [P, DC*D]: row p holds (chunk, out)."""
    return np.ascontiguousarray(
        wt.reshape(DC, P, D).transpose(1, 0, 2).reshape(P, DC * D)
    )


def _prep_shared(Wq, bq, Wk, Wv, bv, Wo, bo):
    return {
        "wq8": _f8(_wtile(np.ascontiguousarray(Wq.T)), WS),
        "wk8": _f8(_wtile(np.ascontiguousarray(Wk.T)), WS),
        "wvb": _wtile(np.ascontiguousarray(Wv.T)).astype(_BF16),
        "wob": _wtile(np.ascontiguousarray(Wo.T * 0.5)).astype(_BF16),
        "bqs": _fm(SCALE * bq),
        "bos": _fm(bo),
        "tbv": _fm(float(T) * bv),
        "bvr": np.ascontiguousarray(bv.reshape(1, D)).astype(_BF16),
    }


def _run(in_maps, trace=False, debug=False, cores=None, **kw):
    from concourse.bass_utils import run_bass_kernel_spmd

    nc = _get_program(debug)
    if cores is None:
        cores = list(range(NCORES))
    return run_bass_kernel_spmd(nc, in_maps, cores, trace=trace, **kw)


def kernel(x, Wq, bq, Wk, Wv, bv, Wo, bo):
    x = np.asarray(x, dtype=np.float32)
    assert x.shape == (B, T, D), x.shape
    shared = _prep_shared(
        np.asarray(Wq, np.float32), np.asarray(bq, np.float32),
        np.asarray(Wk, np.float32), np.asarray(Wv, np.float32),
        np.asarray(bv, np.float32), np.asarray(Wo, np.float32),
        np.asarray(bo, np.float32),
    )
    in_maps = []
    for b in range(B):
        m = dict(shared)
        xb = x[b]
        xbT = np.ascontiguousarray(xb.T)
        m["xT8"] = _f8(
            xbT.reshape(DC, P, NTT, TT).transpose(2, 1, 0, 3).reshape(NTT, P, DC * TT),
            XS,
        )
        m["xt8"] = _f8(
            xb.reshape(NG, P, D).transpose(1, 0, 2).reshape(P, NG * D), XS
        )
        m["csx"] = _fm(xb.sum(axis=0))
        in_maps.append(m)

    res = _run(in_maps)
    out = np.empty((B, T, D), np.float32)
    for b in range(B):
        out[b] = res.results[b]["yT"].astype(np.float32).T
    return out
